# revision 8
# baseline (speedup 1.0000x reference)
"""Argmax-pivoted Gram-Schmidt (BaseSAE resample) on 8 Trainium2 NeuronCores.

Math: the reference (argmax-pivoted MGS with rank-1 deflation of all 8192
candidates) is exactly pivoted Cholesky on the Gram matrix G = X @ X.T:
  d_r      = ||x_r||^2 - sum_j C[r,j]^2          (residual norms)
  p_i      = argmax_r d_r ;  vn_i = sqrt(d_p)
  C[r,i]   = (G[r,p] - sum_{j<i} C[r,j]*C[p,j]) / vn_i
  V        = Lp^{-1} @ X[pivots]   with Lp = C[pivots,:] lower-triangular
d is tracked with Kahan compensation so the device pivot sequence matches
the fp32 reference (validated: min argmax top-2 margin 0.0063 on this
input; compensated downdate error ~1e-4).

Distribution (8 cores):
  Gram phase: core j computes B_j = X @ X_j^T  [8192,1024] (its column
    block of G) -> every core locally owns G[p, local rows] for ANY p.
  Iteration phase (128 sequential steps): one 132-float AllGather per step
    carrying [local max d, local argmax idx, C[q_local, :]] -- winner
    selection and coefficient-row broadcast fused into a single collective.
  Reconstruction: column-sharded forward substitution (512 cols/core),
    host concatenates the 8 blocks.
"""

import sys

import numpy as np

for _p in ("/root/.axon_site", "/root/.axon_site/_ro/trn_rl_repo", "/opt/trn_rl_repo"):
    if _p not in sys.path:
        sys.path.append(_p)

from concourse import bass, bacc, tile, mybir  # noqa: E402
from concourse.bass_utils import run_bass_kernel_spmd  # noqa: E402

F32 = mybir.dt.float32
U32 = mybir.dt.uint32
I32 = mybir.dt.int32
ET = mybir.EngineType
ALU = mybir.AluOpType
ACTF = mybir.ActivationFunctionType

N, D, K, NCORE = 8192, 4096, 128, 8


def _build(N=N, D=D, K=K, debug=False):
    NCORE = 8
    LR = N // NCORE
    LC = D // NCORE
    SLOT = K + 4
    KT = D // 128
    nc = bacc.Bacc("TRN2", target_bir_lowering=False, debug=False, num_devices=NCORE)

    xtt = nc.dram_tensor("xtt", [KT * (N // 128) * 128, 128], F32, kind="ExternalInput").ap()
    xtl = nc.dram_tensor("xtl", [D, LR], F32, kind="ExternalInput").ap()
    xcols = nc.dram_tensor("xcols", [N, LC], F32, kind="ExternalInput").ap()
    d0 = nc.dram_tensor("d0", [1, LR], F32, kind="ExternalInput").ap()
    vout = nc.dram_tensor("vout", [K, LC], F32, kind="ExternalOutput").ap()
    vnso = nc.dram_tensor("vnso", [1, K], F32, kind="ExternalOutput").ap()
    pivo = nc.dram_tensor("pivo", [1, K], I32, kind="ExternalOutput").ap()

    bd = nc.dram_tensor("bd", [N, LR], F32)  # Gram block, internal HBM
    if debug:
        bdo = nc.dram_tensor("bdo", [N, LR], F32, kind="ExternalOutput").ap()
        cto = nc.dram_tensor("cto", [K, LR], F32, kind="ExternalOutput").ap()

    with tile.TileContext(nc) as tc:
        with (
            tc.tile_pool(name="pers", bufs=1) as pers,
            tc.tile_pool(name="dram", bufs=2, space="DRAM") as dpool,
        ):
            # persistent state
            ct = pers.tile([K, LR], F32)  # C^T: row i = column i of C (local rows)
            lpt = pers.tile([K, K], F32)  # Lp^T: col i = winner coeff col of iter i
            vsb = pers.tile([K, LC], F32)  # V column block
            psb = pers.tile([K, LC], F32)  # X[pivots] column block
            tp3 = pers.tile([K, LC], F32)  # phase-3 scratch
            da = pers.tile([1, LR], F32)
            db = pers.tile([1, LR], F32)
            comp = pers.tile([1, LR], F32)
            brow = pers.tile([1, LR], F32)  # also reused as Kahan scratch r
            cnew = pers.tile([1, LR], F32)
            c2 = pers.tile([1, LR], F32)  # also reused as Kahan scratch s
            gcol = pers.tile([K, 1], F32)  # own-candidate C column for AG
            vns = pers.tile([1, K], F32)
            ivn = pers.tile([1, K], F32)
            piv = pers.tile([1, K], I32)
            scan = pers.tile([1, 2 * NCORE], F32)
            maxl = pers.tile([1, 8], F32)
            idxl = pers.tile([1, 8], U32)
            idxf = pers.tile([1, 1], F32)
            max8 = pers.tile([1, 8], F32)
            idx8 = pers.tile([1, 8], U32)
            qu = pers.tile([1, 1], U32)

            nc.vector.memset(ct[:], 0.0)
            nc.vector.memset(lpt[:], 0.0)
            nc.vector.memset(vsb[:], 0.0)
            nc.vector.memset(comp[:], 0.0)
            nc.vector.memset(piv[:], 0)
            nc.sync.dma_start(out=da[:], in_=d0[:])

            # ---------------- Phase 1: Gram block B_j = X @ X_j^T ----------------
            with (
                tc.tile_pool(name="gram", bufs=1) as gpool,
                tc.tile_pool(name="lhs", bufs=2) as lpool,
                tc.tile_pool(name="bout", bufs=2) as bpool,
                tc.tile_pool(name="gps", bufs=2, space="PSUM") as gps,
            ):
                rhs = gpool.tile([128, KT * LR], F32)  # resident X_j^T k-tiles
                for k in range(KT):
                    nc.sync.dma_start(
                        out=rhs[:, k * LR : (k + 1) * LR],
                        in_=xtl[k * 128 : (k + 1) * 128, :],
                    )
                for m in range(N // 128):
                    # lhsT for this m-block: two k-halves, double-buffered
                    lts = []
                    nhalf = 2 if KT > 1 else 1
                    kh = KT // nhalf  # k-tiles per half
                    for half in range(nhalf):
                        lt = lpool.tile([128, kh * 128], F32, tag="lt")
                        for kk in range(kh):
                            k = half * kh + kk
                            t0 = (k * (N // 128) + m) * 128
                            nc.sync.dma_start(
                                out=lt[:, kk * 128 : (kk + 1) * 128],
                                in_=xtt[t0 : t0 + 128, :],
                            )
                        lts.append(lt)
                    HH = LR // 2
                    ps0 = gps.tile([128, HH], F32, tag="ps0")
                    ps1 = gps.tile([128, HH], F32, tag="ps1")
                    for k in range(KT):
                        lt = lts[k // kh]
                        lslice = lt[:, (k % kh) * 128 : (k % kh + 1) * 128]
                        nc.tensor.matmul(
                            ps0[:], lhsT=lslice, rhs=rhs[:, k * LR : k * LR + HH],
                            start=(k == 0), stop=(k == KT - 1),
                        )
                        nc.tensor.matmul(
                            ps1[:], lhsT=lslice, rhs=rhs[:, k * LR + HH : (k + 1) * LR],
                            start=(k == 0), stop=(k == KT - 1),
                        )
                    bsb = bpool.tile([128, LR], F32, tag="bsb")
                    nc.vector.tensor_copy(out=bsb[:, 0:HH], in_=ps0[:])
                    nc.vector.tensor_copy(out=bsb[:, HH:LR], in_=ps1[:])
                    nc.sync.dma_start(out=bd[m * 128 : (m + 1) * 128, :], in_=bsb[:])

            # ---------------- Phase 2: 128 pivoted-Cholesky iterations ----------------
            with tc.tile_pool(name="cps", bufs=2, space="PSUM") as cpool:
                for i in range(K):
                    d_cur, d_nxt = (da, db) if i % 2 == 0 else (db, da)

                    # local argmax of d
                    nc.vector.max(out=maxl[:], in_=d_cur[:])
                    nc.vector.max_index(out=idxl[:], in_max=maxl[:], in_values=d_cur[:])
                    nc.vector.tensor_copy(out=idxf[:], in_=idxl[0:1, 0:1])
                    # own-candidate coefficient column (rows >= i are still zero)
                    qs_regs = nc.alloc_registers(f"qs{i}", engines=[ET.DVE])
                    nc.regs_load(qs_regs, idxl[0:1, 0:1])
                    qs = nc.snap(qs_regs, donate=True, min_val=0, max_val=LR - 1)
                    nc.vector.tensor_copy(out=gcol[:], in_=ct[:, bass.ds(qs, 1)])

                    # pack + AllGather
                    agi = dpool.tile([SLOT, 1], F32, tag="agi")
                    nc.sync.dma_start(out=agi[0:1, 0:1], in_=maxl[0:1, 0:1])
                    nc.sync.dma_start(out=agi[1:2, 0:1], in_=idxf[:])
                    nc.sync.dma_start(out=agi[2 : 2 + K, 0:1], in_=gcol[:])
                    ago = dpool.tile([NCORE * SLOT, 1], F32, tag="ago")
                    nc.gpsimd.collective_compute(
                        "AllGather",
                        ALU.bypass,
                        ins=[agi[:].opt()],
                        outs=[ago[:].opt()],
                        replica_groups=[list(range(NCORE))],
                    )

                    # winner pick
                    nc.sync.dma_start(
                        out=scan[:],
                        in_=ago[:].rearrange("(a b) c -> a (b c)", b=SLOT)[:, 0:2],
                    )
                    nc.vector.max(out=max8[:], in_=scan[0:1, 0 : 2 * NCORE : 2])
                    nc.vector.max_index(
                        out=idx8[:], in_max=max8[:], in_values=scan[0:1, 0 : 2 * NCORE : 2]
                    )
                    nc.scalar.activation(vns[0:1, i : i + 1], max8[0:1, 0:1], ACTF.Sqrt)
                    nc.vector.reciprocal(ivn[0:1, i : i + 1], vns[0:1, i : i + 1])

                    o_regs = nc.alloc_registers(f"o{i}", engines=[ET.SP, ET.DVE])
                    nc.regs_load(o_regs, idx8[0:1, 0:1])
                    o_sv = nc.snap(o_regs, donate=True, min_val=0, max_val=NCORE - 1)
                    nc.vector.tensor_copy(
                        out=qu[:], in_=scan[0:1, bass.ds(o_sv * 2 + 1, 1)]
                    )
                    q_regs = nc.alloc_registers(f"q{i}", engines=[ET.SP])
                    nc.reg_load(q_regs, qu[0:1, 0:1])
                    q_sv = nc.snap(q_regs, donate=True, min_val=0, max_val=LR - 1)
                    p_sv = o_sv * LR + q_sv
                    nc.sync.reg_save(out=piv[0:1, i : i + 1], in_=p_sv)

                    # winner coefficient column, Gram row, pivot X-row (phase-3)
                    gwin = pers.tile([K, 1], F32, tag="gwin")
                    nc.sync.dma_start(out=gwin[:], in_=ago[bass.ds(o_sv * SLOT + 2, K), 0:1])
                    nc.sync.dma_start(out=brow[:], in_=bd[bass.ds(p_sv, 1), :])
                    nc.sync.dma_start(out=psb[i : i + 1, :], in_=xcols[bass.ds(p_sv, 1), :])
                    nc.vector.tensor_copy(out=lpt[:, i : i + 1], in_=gwin[:])

                    # c_new = (brow - C^T-correction) * ivn
                    if i > 0:
                        HH2 = LR // 2
                        cp0 = cpool.tile([1, HH2], F32, tag="cp0")
                        cp1 = cpool.tile([1, HH2], F32, tag="cp1")
                        nc.tensor.matmul(
                            cp0[:], lhsT=gwin[:K, :], rhs=ct[:, 0:HH2], start=True, stop=True
                        )
                        nc.tensor.matmul(
                            cp1[:], lhsT=gwin[:K, :], rhs=ct[:, HH2:LR], start=True, stop=True
                        )
                        nc.vector.tensor_tensor(
                            out=cnew[0:1, 0:HH2], in0=brow[0:1, 0:HH2], in1=cp0[:],
                            op=ALU.subtract,
                        )
                        nc.vector.tensor_tensor(
                            out=cnew[0:1, HH2:LR], in0=brow[0:1, HH2:LR], in1=cp1[:],
                            op=ALU.subtract,
                        )
                        nc.vector.tensor_scalar(
                            cnew[:], cnew[:], ivn[0:1, i : i + 1], None, ALU.mult
                        )
                    else:
                        nc.vector.tensor_scalar(
                            cnew[:], brow[:], ivn[0:1, i : i + 1], None, ALU.mult
                        )

                    # Kahan-compensated downdate: d -= c_new^2
                    # s = c2 + comp ; t = d - s ; r = d - t ; comp = s - r
                    nc.vector.tensor_tensor(out=c2[:], in0=cnew[:], in1=cnew[:], op=ALU.mult)
                    nc.vector.tensor_tensor(out=c2[:], in0=c2[:], in1=comp[:], op=ALU.add)
                    nc.vector.tensor_tensor(out=d_nxt[:], in0=d_cur[:], in1=c2[:], op=ALU.subtract)
                    nc.vector.tensor_tensor(out=brow[:], in0=d_cur[:], in1=d_nxt[:], op=ALU.subtract)
                    nc.vector.tensor_tensor(out=comp[:], in0=c2[:], in1=brow[:], op=ALU.subtract)

                    # store column i of C (row i of C^T)
                    nc.sync.dma_start(out=ct[i : i + 1, :], in_=cnew[:])

            # ---------------- Phase 3: forward substitution V = Lp^{-1} P ----------------
            # All row-i work happens at partition 0 (PE psum base must be 0/32/64);
            # finished rows are DMA'd to partition i of vsb for use as matmul rhs.
            with (
                tc.tile_pool(name="vps", bufs=2, space="PSUM") as vpool,
                tc.tile_pool(name="p3", bufs=4) as p3pool,
            ):
                for i in range(K):
                    prow = p3pool.tile([1, LC], F32, tag="prow")
                    nc.sync.dma_start(out=prow[:], in_=psb[i : i + 1, :])
                    vrow = p3pool.tile([1, LC], F32, tag="vrow")
                    if i > 0:
                        vps = vpool.tile([1, LC], F32, tag="vps")
                        nc.tensor.matmul(
                            vps[:], lhsT=lpt[:, i : i + 1], rhs=vsb[:],
                            start=True, stop=True,
                        )
                        nc.vector.tensor_tensor(
                            out=vrow[:], in0=prow[:], in1=vps[:], op=ALU.subtract
                        )
                        nc.vector.tensor_scalar(
                            vrow[:], vrow[:], ivn[0:1, i : i + 1], None, ALU.mult
                        )
                    else:
                        nc.vector.tensor_scalar(
                            vrow[:], prow[:], ivn[0:1, 0:1], None, ALU.mult
                        )
                    nc.sync.dma_start(out=vsb[i : i + 1, :], in_=vrow[:])

            if debug:
                nc.sync.dma_start(out=bdo[:], in_=bd[:, :])
                nc.sync.dma_start(out=cto[:], in_=ct[:])
            nc.sync.dma_start(out=vout[:], in_=vsb[:])
            nc.sync.dma_start(out=vnso[:], in_=vns[:])
            nc.sync.dma_start(out=pivo[:], in_=piv[:])

    nc.compile()
    return nc


_NC_CACHE = None


def _get_nc():
    global _NC_CACHE
    if _NC_CACHE is None:
        _NC_CACHE = _build()
    return _NC_CACHE


def run_device(x, nc=None, **kwargs):
    x = np.ascontiguousarray(x, dtype=np.float32)
    n, d = x.shape
    NCORE = 8
    LR = n // NCORE
    LC = d // NCORE
    xt = np.ascontiguousarray(x.T)
    KT, NM = d // 128, n // 128
    xtt = np.ascontiguousarray(
        xt.reshape(KT, 128, NM, 128).transpose(0, 2, 1, 3)
    ).reshape(KT * NM * 128, 128)
    d0 = np.einsum("ij,ij->i", x, x)
    in_maps = []
    for j in range(NCORE):
        in_maps.append(
            {
                "xtt": xtt,
                "xtl": np.ascontiguousarray(xt[:, j * LR : (j + 1) * LR]),
                "xcols": np.ascontiguousarray(x[:, j * LC : (j + 1) * LC]),
                "d0": np.ascontiguousarray(d0[j * LR : (j + 1) * LR]).reshape(1, LR),
            }
        )
    if nc is None:
        nc = _get_nc()
    return run_bass_kernel_spmd(nc, in_maps, core_ids=list(range(NCORE)), **kwargs)


def kernel(x_diff):
    out = run_device(x_diff)
    res = out.results
    V = np.concatenate([res[j]["vout"] for j in range(8)], axis=1).astype(np.float32)
    vns = res[0]["vnso"].reshape(-1)
    broken = np.where(vns < 1e-6)[0]
    n_succ = int(broken[0]) if len(broken) else K
    if n_succ < K:
        V[n_succ:] = 0.0
    return V, np.int32(n_succ)


# revision 9
# speedup vs baseline: 1.0902x; 1.0902x over previous
"""Argmax-pivoted Gram-Schmidt (BaseSAE resample) on 8 Trainium2 NeuronCores.

Math: the reference (argmax-pivoted MGS with rank-1 deflation of all 8192
candidates) is exactly pivoted Cholesky on the Gram matrix G = X @ X.T:
  d_r      = ||x_r||^2 - sum_j C[r,j]^2          (residual norms)
  p_i      = argmax_r d_r ;  vn_i = sqrt(d_p)
  C[r,i]   = (G[r,p] - sum_{j<i} C[r,j]*C[p,j]) / vn_i
  V        = Lp^{-1} @ X[pivots]   with Lp = C[pivots,:] lower-triangular
d is tracked with Kahan compensation so the device pivot sequence matches
the fp32 reference (validated: min argmax top-2 margin 0.0063 on this
input; compensated downdate error ~1e-4).

Distribution (8 cores):
  Gram phase: core j computes B_j = X @ X_j^T  [8192,1024] (its column
    block of G) -> every core locally owns G[p, local rows] for ANY p.
  Iteration phase (128 sequential steps): one 132-float AllGather per step
    carrying [local max d, local argmax idx, C[q_local, :]] -- winner
    selection and coefficient-row broadcast fused into a single collective.
  Reconstruction: column-sharded forward substitution (512 cols/core),
    host concatenates the 8 blocks.
"""

import sys

import numpy as np

for _p in ("/root/.axon_site", "/root/.axon_site/_ro/trn_rl_repo", "/opt/trn_rl_repo"):
    if _p not in sys.path:
        sys.path.append(_p)

from concourse import bass, bacc, tile, mybir  # noqa: E402
from concourse.bass_utils import run_bass_kernel_spmd  # noqa: E402

F32 = mybir.dt.float32
U32 = mybir.dt.uint32
I32 = mybir.dt.int32
ET = mybir.EngineType
ALU = mybir.AluOpType
ACTF = mybir.ActivationFunctionType

N, D, K, NCORE = 8192, 4096, 128, 8


def _build(N=N, D=D, K=K, debug=False):
    NCORE = 8
    LR = N // NCORE
    LC = D // NCORE
    SLOT = K + 4
    KT = D // 128
    nc = bacc.Bacc("TRN2", target_bir_lowering=False, debug=False, num_devices=NCORE)

    ML = LR // 128  # local m-tiles per core
    xttl = nc.dram_tensor("xttl", [ML * KT * 128, 128], F32, kind="ExternalInput").ap()
    xcols = nc.dram_tensor("xcols", [N, LC], F32, kind="ExternalInput").ap()
    d0 = nc.dram_tensor("d0", [1, LR], F32, kind="ExternalInput").ap()
    vout = nc.dram_tensor("vout", [K, LC], F32, kind="ExternalOutput").ap()
    vnso = nc.dram_tensor("vnso", [1, K], F32, kind="ExternalOutput").ap()
    pivo = nc.dram_tensor("pivo", [1, K], I32, kind="ExternalOutput").ap()

    bd = nc.dram_tensor("bd", [N, LR], F32)  # Gram block, internal HBM
    if debug:
        bdo = nc.dram_tensor("bdo", [N, LR], F32, kind="ExternalOutput").ap()
        cto = nc.dram_tensor("cto", [K, LR], F32, kind="ExternalOutput").ap()

    with tile.TileContext(nc) as tc:
        with (
            tc.tile_pool(name="pers", bufs=1) as pers,
            tc.tile_pool(name="dram", bufs=2, space="DRAM") as dpool,
        ):
            # persistent state
            ct = pers.tile([K, LR], F32)  # C^T: row i = column i of C (local rows)
            lpt = pers.tile([K, K], F32)  # Lp^T: col i = winner coeff col of iter i
            vsb = pers.tile([K, LC], F32)  # V column block
            psb = pers.tile([K, LC], F32)  # X[pivots] column block
            tp3 = pers.tile([K, LC], F32)  # phase-3 scratch
            da = pers.tile([1, LR], F32)
            db = pers.tile([1, LR], F32)
            comp = pers.tile([1, LR], F32)
            brow = pers.tile([1, LR], F32)  # also reused as Kahan scratch r
            cnew = pers.tile([1, LR], F32)
            c2 = pers.tile([1, LR], F32)  # also reused as Kahan scratch s
            gcol = pers.tile([K, 1], F32)  # own-candidate C column for AG
            vns = pers.tile([1, K], F32)
            ivn = pers.tile([1, K], F32)
            piv = pers.tile([1, K], I32)
            scan = pers.tile([1, 2 * NCORE], F32)
            maxl = pers.tile([1, 8], F32)
            idxl = pers.tile([1, 8], U32)
            idxf = pers.tile([1, 1], F32)
            max8 = pers.tile([1, 8], F32)
            idx8 = pers.tile([1, 8], U32)
            qu = pers.tile([1, 1], U32)

            nc.vector.memset(ct[:], 0.0)
            nc.vector.memset(lpt[:], 0.0)
            nc.vector.memset(vsb[:], 0.0)
            nc.vector.memset(comp[:], 0.0)
            nc.vector.memset(piv[:], 0)
            nc.sync.dma_start(out=da[:], in_=d0[:])

            # ---------------- Phase 1: Gram block B_j = X @ X_j^T ----------------
            with (
                tc.tile_pool(name="gram", bufs=1) as gpool,
                tc.tile_pool(name="lhs", bufs=2) as lpool,
                tc.tile_pool(name="bout", bufs=2) as bpool,
                tc.tile_pool(name="gps", bufs=2, space="PSUM") as gps,
            ):
                # gather all cores' lhsT tiles: global tile (k, m) at row (m*KT+k)*128
                xin = dpool.tile([ML * KT * 128, 128], F32)
                nc.sync.dma_start(out=xin[:], in_=xttl[:])
                xtg = dpool.tile([NCORE * ML * KT * 128, 128], F32)
                nc.gpsimd.collective_compute(
                    "AllGather",
                    ALU.bypass,
                    ins=[xin[:].opt()],
                    outs=[xtg[:].opt()],
                    replica_groups=[list(range(NCORE))],
                )
                rhs = gpool.tile([128, KT * LR], F32)  # resident X_j^T k-tiles
                for k in range(KT):
                    for ml in range(ML):
                        t0 = (ml * KT + k) * 128
                        nc.sync.dma_start(
                            out=rhs[:, k * LR + ml * 128 : k * LR + (ml + 1) * 128],
                            in_=xttl[t0 : t0 + 128, :],
                        )
                for m in range(N // 128):
                    # lhsT for this m-block: two k-halves, double-buffered
                    lts = []
                    nhalf = 2 if KT > 1 else 1
                    kh = KT // nhalf  # k-tiles per half
                    for half in range(nhalf):
                        lt = lpool.tile([128, kh * 128], F32, tag="lt")
                        for kk in range(kh):
                            k = half * kh + kk
                            t0 = (m * KT + k) * 128
                            nc.sync.dma_start(
                                out=lt[:, kk * 128 : (kk + 1) * 128],
                                in_=xtg[t0 : t0 + 128, :],
                            )
                        lts.append(lt)
                    HH = LR // 2
                    ps0 = gps.tile([128, HH], F32, tag="ps0")
                    ps1 = gps.tile([128, HH], F32, tag="ps1")
                    for k in range(KT):
                        lt = lts[k // kh]
                        lslice = lt[:, (k % kh) * 128 : (k % kh + 1) * 128]
                        nc.tensor.matmul(
                            ps0[:], lhsT=lslice, rhs=rhs[:, k * LR : k * LR + HH],
                            start=(k == 0), stop=(k == KT - 1),
                        )
                        nc.tensor.matmul(
                            ps1[:], lhsT=lslice, rhs=rhs[:, k * LR + HH : (k + 1) * LR],
                            start=(k == 0), stop=(k == KT - 1),
                        )
                    bsb = bpool.tile([128, LR], F32, tag="bsb")
                    nc.vector.tensor_copy(out=bsb[:, 0:HH], in_=ps0[:])
                    nc.vector.tensor_copy(out=bsb[:, HH:LR], in_=ps1[:])
                    nc.sync.dma_start(out=bd[m * 128 : (m + 1) * 128, :], in_=bsb[:])

            # ---------------- Phase 2: 128 pivoted-Cholesky iterations ----------------
            with tc.tile_pool(name="cps", bufs=2, space="PSUM") as cpool:
                for i in range(K):
                    d_cur, d_nxt = (da, db) if i % 2 == 0 else (db, da)

                    # local argmax of d
                    nc.vector.max(out=maxl[:], in_=d_cur[:])
                    nc.vector.max_index(out=idxl[:], in_max=maxl[:], in_values=d_cur[:])
                    nc.vector.tensor_copy(out=idxf[:], in_=idxl[0:1, 0:1])
                    # own-candidate coefficient column (rows >= i are still zero)
                    qs_regs = nc.alloc_registers(f"qs{i}", engines=[ET.DVE])
                    nc.regs_load(qs_regs, idxl[0:1, 0:1])
                    qs = nc.snap(qs_regs, donate=True, min_val=0, max_val=LR - 1)
                    nc.vector.tensor_copy(out=gcol[:], in_=ct[:, bass.ds(qs, 1)])

                    # pack + AllGather
                    agi = dpool.tile([SLOT, 1], F32, tag="agi")
                    nc.sync.dma_start(out=agi[0:1, 0:1], in_=maxl[0:1, 0:1])
                    nc.sync.dma_start(out=agi[1:2, 0:1], in_=idxf[:])
                    nc.sync.dma_start(out=agi[2 : 2 + K, 0:1], in_=gcol[:])
                    ago = dpool.tile([NCORE * SLOT, 1], F32, tag="ago")
                    nc.gpsimd.collective_compute(
                        "AllGather",
                        ALU.bypass,
                        ins=[agi[:].opt()],
                        outs=[ago[:].opt()],
                        replica_groups=[list(range(NCORE))],
                    )

                    # winner pick
                    nc.sync.dma_start(
                        out=scan[:],
                        in_=ago[:].rearrange("(a b) c -> a (b c)", b=SLOT)[:, 0:2],
                    )
                    nc.vector.max(out=max8[:], in_=scan[0:1, 0 : 2 * NCORE : 2])
                    nc.vector.max_index(
                        out=idx8[:], in_max=max8[:], in_values=scan[0:1, 0 : 2 * NCORE : 2]
                    )
                    nc.scalar.activation(vns[0:1, i : i + 1], max8[0:1, 0:1], ACTF.Sqrt)
                    nc.vector.reciprocal(ivn[0:1, i : i + 1], vns[0:1, i : i + 1])

                    o_regs = nc.alloc_registers(f"o{i}", engines=[ET.SP, ET.DVE])
                    nc.regs_load(o_regs, idx8[0:1, 0:1])
                    o_sv = nc.snap(o_regs, donate=True, min_val=0, max_val=NCORE - 1)
                    nc.vector.tensor_copy(
                        out=qu[:], in_=scan[0:1, bass.ds(o_sv * 2 + 1, 1)]
                    )
                    q_regs = nc.alloc_registers(f"q{i}", engines=[ET.SP])
                    nc.reg_load(q_regs, qu[0:1, 0:1])
                    q_sv = nc.snap(q_regs, donate=True, min_val=0, max_val=LR - 1)
                    p_sv = o_sv * LR + q_sv
                    nc.sync.reg_save(out=piv[0:1, i : i + 1], in_=p_sv)

                    # winner coefficient column, Gram row, pivot X-row (phase-3)
                    gwin = pers.tile([K, 1], F32, tag="gwin")
                    nc.sync.dma_start(out=gwin[:], in_=ago[bass.ds(o_sv * SLOT + 2, K), 0:1])
                    nc.sync.dma_start(out=brow[:], in_=bd[bass.ds(p_sv, 1), :])
                    nc.sync.dma_start(out=psb[i : i + 1, :], in_=xcols[bass.ds(p_sv, 1), :])
                    nc.vector.tensor_copy(out=lpt[:, i : i + 1], in_=gwin[:])

                    # c_new = (brow - C^T-correction) * ivn
                    if i > 0:
                        HH2 = LR // 2
                        cp0 = cpool.tile([1, HH2], F32, tag="cp0")
                        cp1 = cpool.tile([1, HH2], F32, tag="cp1")
                        nc.tensor.matmul(
                            cp0[:], lhsT=gwin[:K, :], rhs=ct[:, 0:HH2], start=True, stop=True
                        )
                        nc.tensor.matmul(
                            cp1[:], lhsT=gwin[:K, :], rhs=ct[:, HH2:LR], start=True, stop=True
                        )
                        nc.vector.tensor_tensor(
                            out=cnew[0:1, 0:HH2], in0=brow[0:1, 0:HH2], in1=cp0[:],
                            op=ALU.subtract,
                        )
                        nc.vector.tensor_tensor(
                            out=cnew[0:1, HH2:LR], in0=brow[0:1, HH2:LR], in1=cp1[:],
                            op=ALU.subtract,
                        )
                        nc.vector.tensor_scalar(
                            cnew[:], cnew[:], ivn[0:1, i : i + 1], None, ALU.mult
                        )
                    else:
                        nc.vector.tensor_scalar(
                            cnew[:], brow[:], ivn[0:1, i : i + 1], None, ALU.mult
                        )

                    # Kahan-compensated downdate: d -= c_new^2
                    # s = c2 + comp ; t = d - s ; r = d - t ; comp = s - r
                    nc.vector.tensor_tensor(out=c2[:], in0=cnew[:], in1=cnew[:], op=ALU.mult)
                    nc.vector.tensor_tensor(out=c2[:], in0=c2[:], in1=comp[:], op=ALU.add)
                    nc.vector.tensor_tensor(out=d_nxt[:], in0=d_cur[:], in1=c2[:], op=ALU.subtract)
                    nc.vector.tensor_tensor(out=brow[:], in0=d_cur[:], in1=d_nxt[:], op=ALU.subtract)
                    nc.vector.tensor_tensor(out=comp[:], in0=c2[:], in1=brow[:], op=ALU.subtract)

                    # store column i of C (row i of C^T)
                    nc.sync.dma_start(out=ct[i : i + 1, :], in_=cnew[:])

            # ---------------- Phase 3: forward substitution V = Lp^{-1} P ----------------
            # All row-i work happens at partition 0 (PE psum base must be 0/32/64);
            # finished rows are DMA'd to partition i of vsb for use as matmul rhs.
            with (
                tc.tile_pool(name="vps", bufs=2, space="PSUM") as vpool,
                tc.tile_pool(name="p3", bufs=4) as p3pool,
            ):
                for i in range(K):
                    prow = p3pool.tile([1, LC], F32, tag="prow")
                    nc.sync.dma_start(out=prow[:], in_=psb[i : i + 1, :])
                    vrow = p3pool.tile([1, LC], F32, tag="vrow")
                    if i > 0:
                        vps = vpool.tile([1, LC], F32, tag="vps")
                        nc.tensor.matmul(
                            vps[:], lhsT=lpt[:, i : i + 1], rhs=vsb[:],
                            start=True, stop=True,
                        )
                        nc.vector.tensor_tensor(
                            out=vrow[:], in0=prow[:], in1=vps[:], op=ALU.subtract
                        )
                        nc.vector.tensor_scalar(
                            vrow[:], vrow[:], ivn[0:1, i : i + 1], None, ALU.mult
                        )
                    else:
                        nc.vector.tensor_scalar(
                            vrow[:], prow[:], ivn[0:1, 0:1], None, ALU.mult
                        )
                    nc.sync.dma_start(out=vsb[i : i + 1, :], in_=vrow[:])

            if debug:
                nc.sync.dma_start(out=bdo[:], in_=bd[:, :])
                nc.sync.dma_start(out=cto[:], in_=ct[:])
            nc.sync.dma_start(out=vout[:], in_=vsb[:])
            nc.sync.dma_start(out=vnso[:], in_=vns[:])
            nc.sync.dma_start(out=pivo[:], in_=piv[:])

    nc.compile()
    return nc


_NC_CACHE = None


def _get_nc():
    global _NC_CACHE
    if _NC_CACHE is None:
        _NC_CACHE = _build()
    return _NC_CACHE


def run_device(x, nc=None, **kwargs):
    x = np.ascontiguousarray(x, dtype=np.float32)
    n, d = x.shape
    NCORE = 8
    LR = n // NCORE
    LC = d // NCORE
    xt = np.ascontiguousarray(x.T)
    KT, ML = d // 128, LR // 128
    d0 = np.einsum("ij,ij->i", x, x)
    in_maps = []
    for j in range(NCORE):
        blk = xt[:, j * LR : (j + 1) * LR]  # [d, LR]
        xttl = np.ascontiguousarray(
            blk.reshape(KT, 128, ML, 128).transpose(2, 0, 1, 3)
        ).reshape(ML * KT * 128, 128)
        in_maps.append(
            {
                "xttl": xttl,
                "xcols": np.ascontiguousarray(x[:, j * LC : (j + 1) * LC]),
                "d0": np.ascontiguousarray(d0[j * LR : (j + 1) * LR]).reshape(1, LR),
            }
        )
    if nc is None:
        nc = _get_nc()
    return run_bass_kernel_spmd(nc, in_maps, core_ids=list(range(NCORE)), **kwargs)


def kernel(x_diff):
    out = run_device(x_diff)
    res = out.results
    V = np.concatenate([res[j]["vout"] for j in range(8)], axis=1).astype(np.float32)
    vns = res[0]["vnso"].reshape(-1)
    broken = np.where(vns < 1e-6)[0]
    n_succ = int(broken[0]) if len(broken) else K
    if n_succ < K:
        V[n_succ:] = 0.0
    return V, np.int32(n_succ)


# revision 10
# speedup vs baseline: 1.3703x; 1.2569x over previous
"""Argmax-pivoted Gram-Schmidt (BaseSAE resample) on 8 Trainium2 NeuronCores.

Math: the reference (argmax-pivoted MGS with rank-1 deflation of all 8192
candidates) is exactly pivoted Cholesky on the Gram matrix G = X @ X.T:
  d_r      = ||x_r||^2 - sum_j C[r,j]^2          (residual norms)
  p_i      = argmax_r d_r ;  vn_i = sqrt(d_p)
  C[r,i]   = (G[r,p] - sum_{j<i} C[r,j]*C[p,j]) / vn_i
  V        = Lp^{-1} @ X[pivots]   with Lp = C[pivots,:] lower-triangular
d is tracked with Kahan compensation so the device pivot sequence matches
the fp32 reference (validated: min argmax top-2 margin 0.0063 on this
input; compensated downdate error ~1e-4).

Distribution (8 cores):
  Gram phase: core j computes B_j = X @ X_j^T  [8192,1024] (its column
    block of G) -> every core locally owns G[p, local rows] for ANY p.
  Iteration phase (128 sequential steps): one 132-float AllGather per step
    carrying [local max d, local argmax idx, C[q_local, :]] -- winner
    selection and coefficient-row broadcast fused into a single collective.
  Reconstruction: column-sharded forward substitution (512 cols/core),
    host concatenates the 8 blocks.
"""

import sys

import numpy as np

for _p in ("/root/.axon_site", "/root/.axon_site/_ro/trn_rl_repo", "/opt/trn_rl_repo"):
    if _p not in sys.path:
        sys.path.append(_p)

from concourse import bass, bacc, tile, mybir  # noqa: E402
from concourse.bass_utils import run_bass_kernel_spmd  # noqa: E402

F32 = mybir.dt.float32
U32 = mybir.dt.uint32
I32 = mybir.dt.int32
ET = mybir.EngineType
ALU = mybir.AluOpType
ACTF = mybir.ActivationFunctionType

N, D, K, NCORE = 8192, 4096, 128, 8


def _build(N=N, D=D, K=K, debug=False):
    NCORE = 8
    LR = N // NCORE
    LC = D // NCORE
    SLOT = K + 4
    KT = D // 128
    nc = bacc.Bacc("TRN2", target_bir_lowering=False, debug=False, num_devices=NCORE)

    ML = LR // 128  # local m-tiles per core
    xttl = nc.dram_tensor("xttl", [ML * KT * 128, 128], F32, kind="ExternalInput").ap()
    xcols = nc.dram_tensor("xcols", [N, LC], F32, kind="ExternalInput").ap()
    d0 = nc.dram_tensor("d0", [1, LR], F32, kind="ExternalInput").ap()
    vout = nc.dram_tensor("vout", [K, LC], F32, kind="ExternalOutput").ap()
    vnso = nc.dram_tensor("vnso", [1, K], F32, kind="ExternalOutput").ap()
    pivo = nc.dram_tensor("pivo", [1, K], I32, kind="ExternalOutput").ap()

    bd = nc.dram_tensor("bd", [N, LR], F32)  # Gram block, internal HBM
    if debug:
        bdo = nc.dram_tensor("bdo", [N, LR], F32, kind="ExternalOutput").ap()
        cto = nc.dram_tensor("cto", [K, LR], F32, kind="ExternalOutput").ap()

    with tile.TileContext(nc) as tc:
        with (
            tc.tile_pool(name="pers", bufs=1) as pers,
            tc.tile_pool(name="dram", bufs=2, space="DRAM") as dpool,
        ):
            # persistent state
            ct = pers.tile([K, LR], F32)  # C^T: row i = column i of C (local rows)
            lpt = pers.tile([K, K], F32)  # Lp^T: col i = winner coeff col of iter i
            vsb = pers.tile([K, LC], F32)  # V column block
            psb = pers.tile([K, LC], F32)  # X[pivots] column block
            tp3 = pers.tile([K, LC], F32)  # phase-3 scratch
            da = pers.tile([1, LR], F32)
            db = pers.tile([1, LR], F32)
            comp = pers.tile([1, LR], F32)
            brow = pers.tile([1, LR], F32)  # also reused as Kahan scratch r
            cnew = pers.tile([1, LR], F32)
            c2 = pers.tile([1, LR], F32)  # also reused as Kahan scratch s
            gcol = pers.tile([K, 1], F32)  # own-candidate C column for AG
            vns = pers.tile([1, K], F32)
            ivn = pers.tile([1, K], F32)
            piv = pers.tile([1, K], I32)
            scan = pers.tile([1, 2 * NCORE], F32)
            maxl = pers.tile([1, 8], F32)
            idxl = pers.tile([1, 8], U32)
            idxf = pers.tile([1, 1], F32)
            max8 = pers.tile([1, 8], F32)
            idx8 = pers.tile([1, 8], U32)
            qu = pers.tile([1, 1], U32)

            nc.vector.memset(ct[:], 0.0)
            nc.vector.memset(lpt[:], 0.0)
            nc.vector.memset(vsb[:], 0.0)
            nc.vector.memset(comp[:], 0.0)
            nc.vector.memset(piv[:], 0)
            nc.sync.dma_start(out=da[:], in_=d0[:])

            # ---------------- Phase 1: Gram block B_j = X @ X_j^T ----------------
            with (
                tc.tile_pool(name="gram", bufs=1) as gpool,
                tc.tile_pool(name="lhs", bufs=2) as lpool,
                tc.tile_pool(name="bout", bufs=2) as bpool,
                tc.tile_pool(name="gps", bufs=2, space="PSUM") as gps,
            ):
                # gather all cores' lhsT tiles: global tile (k, m) at row (m*KT+k)*128
                xin = dpool.tile([ML * KT * 128, 128], F32)
                nc.sync.dma_start(out=xin[:], in_=xttl[:])
                xtg = dpool.tile([NCORE * ML * KT * 128, 128], F32)
                nc.gpsimd.collective_compute(
                    "AllGather",
                    ALU.bypass,
                    ins=[xin[:].opt()],
                    outs=[xtg[:].opt()],
                    replica_groups=[list(range(NCORE))],
                )
                rhs = gpool.tile([128, KT * LR], F32)  # resident X_j^T k-tiles
                for k in range(KT):
                    for ml in range(ML):
                        t0 = (ml * KT + k) * 128
                        nc.sync.dma_start(
                            out=rhs[:, k * LR + ml * 128 : k * LR + (ml + 1) * 128],
                            in_=xttl[t0 : t0 + 128, :],
                        )
                for m in range(N // 128):
                    # lhsT for this m-block: two k-halves, double-buffered
                    lts = []
                    nhalf = 2 if KT > 1 else 1
                    kh = KT // nhalf  # k-tiles per half
                    for half in range(nhalf):
                        lt = lpool.tile([128, kh * 128], F32, tag="lt")
                        for kk in range(kh):
                            k = half * kh + kk
                            t0 = (m * KT + k) * 128
                            nc.sync.dma_start(
                                out=lt[:, kk * 128 : (kk + 1) * 128],
                                in_=xtg[t0 : t0 + 128, :],
                            )
                        lts.append(lt)
                    HH = LR // 2
                    ps0 = gps.tile([128, HH], F32, tag="ps0")
                    ps1 = gps.tile([128, HH], F32, tag="ps1")
                    for k in range(KT):
                        lt = lts[k // kh]
                        lslice = lt[:, (k % kh) * 128 : (k % kh + 1) * 128]
                        nc.tensor.matmul(
                            ps0[:], lhsT=lslice, rhs=rhs[:, k * LR : k * LR + HH],
                            start=(k == 0), stop=(k == KT - 1),
                        )
                        nc.tensor.matmul(
                            ps1[:], lhsT=lslice, rhs=rhs[:, k * LR + HH : (k + 1) * LR],
                            start=(k == 0), stop=(k == KT - 1),
                        )
                    bsb = bpool.tile([128, LR], F32, tag="bsb")
                    nc.vector.tensor_copy(out=bsb[:, 0:HH], in_=ps0[:])
                    nc.vector.tensor_copy(out=bsb[:, HH:LR], in_=ps1[:])
                    nc.sync.dma_start(out=bd[m * 128 : (m + 1) * 128, :], in_=bsb[:])

            # ---------------- Phase 2: 128 pivoted-Cholesky iterations ----------------
            # Local argmax + AG payload for iteration i are produced at the TAIL of
            # iteration i-1 (right after the d-update lands), so the Kahan scratch
            # ops and the C^T row write stay off the AG critical path.
            def local_argmax(i, d_ap):
                nc.vector.max(out=maxl[:], in_=d_ap)
                nc.vector.max_index(out=idxl[:], in_max=maxl[:], in_values=d_ap)
                # idxf goes into maxl slot 1 so [dmax, idxf] ships as one DMA
                nc.vector.tensor_copy(out=maxl[0:1, 1:2], in_=idxl[0:1, 0:1])
                qs_regs = nc.alloc_registers(f"qs{i}", engines=[ET.DVE])
                nc.regs_load(qs_regs, idxl[0:1, 0:1])
                qs = nc.snap(qs_regs, donate=True, min_val=0, max_val=LR - 1)
                # candidate coefficient column C[q, 0:i] (rows >= i still zero)
                nc.vector.tensor_copy(out=gcol[:], in_=ct[:, bass.ds(qs, 1)])

            with tc.tile_pool(name="cps", bufs=2, space="PSUM") as cpool:
                local_argmax(0, da[:])
                for i in range(K):
                    d_cur, d_nxt = (da, db) if i % 2 == 0 else (db, da)

                    # pack + AllGather (argmax/gcol were produced last iteration)
                    agi = dpool.tile([SLOT, 1], F32, tag="agi")
                    nc.sync.dma_start(out=agi[0:2, 0:1], in_=maxl[0:1, 0:2])
                    nc.sync.dma_start(out=agi[2 : 2 + K, 0:1], in_=gcol[:])
                    ago = dpool.tile([NCORE * SLOT, 1], F32, tag="ago")
                    nc.gpsimd.collective_compute(
                        "AllGather",
                        ALU.bypass,
                        ins=[agi[:].opt()],
                        outs=[ago[:].opt()],
                        replica_groups=[list(range(NCORE))],
                    )

                    # winner pick
                    nc.sync.dma_start(
                        out=scan[:],
                        in_=ago[:].rearrange("(a b) c -> a (b c)", b=SLOT)[:, 0:2],
                    )
                    nc.vector.max(out=max8[:], in_=scan[0:1, 0 : 2 * NCORE : 2])
                    nc.vector.max_index(
                        out=idx8[:], in_max=max8[:], in_values=scan[0:1, 0 : 2 * NCORE : 2]
                    )
                    nc.scalar.activation(vns[0:1, i : i + 1], max8[0:1, 0:1], ACTF.Sqrt)
                    nc.vector.reciprocal(ivn[0:1, i : i + 1], vns[0:1, i : i + 1])

                    o_regs = nc.alloc_registers(f"o{i}", engines=[ET.SP, ET.DVE])
                    nc.regs_load(o_regs, idx8[0:1, 0:1])
                    o_sv = nc.snap(o_regs, donate=True, min_val=0, max_val=NCORE - 1)
                    nc.vector.tensor_copy(
                        out=qu[:], in_=scan[0:1, bass.ds(o_sv * 2 + 1, 1)]
                    )
                    q_regs = nc.alloc_registers(f"q{i}", engines=[ET.SP])
                    nc.reg_load(q_regs, qu[0:1, 0:1])
                    q_sv = nc.snap(q_regs, donate=True, min_val=0, max_val=LR - 1)
                    p_sv = o_sv * LR + q_sv
                    nc.sync.reg_save(out=piv[0:1, i : i + 1], in_=p_sv)

                    # winner coefficient column, Gram row, pivot X-row (phase-3)
                    gwin = pers.tile([K, 1], F32, tag="gwin")
                    nc.sync.dma_start(out=gwin[:], in_=ago[bass.ds(o_sv * SLOT + 2, K), 0:1])
                    nc.sync.dma_start(out=brow[:], in_=bd[bass.ds(p_sv, 1), :])
                    nc.sync.dma_start(out=psb[i : i + 1, :], in_=xcols[bass.ds(p_sv, 1), :])
                    nc.vector.tensor_copy(out=lpt[:, i : i + 1], in_=gwin[:])

                    # c_new = (brow - C^T-correction) * ivn
                    if i > 0:
                        HH2 = LR // 2
                        cp0 = cpool.tile([1, HH2], F32, tag="cp0")
                        cp1 = cpool.tile([1, HH2], F32, tag="cp1")
                        nc.tensor.matmul(
                            cp0[:], lhsT=gwin[:K, :], rhs=ct[:, 0:HH2], start=True, stop=True
                        )
                        nc.tensor.matmul(
                            cp1[:], lhsT=gwin[:K, :], rhs=ct[:, HH2:LR], start=True, stop=True
                        )
                        nc.vector.tensor_tensor(
                            out=cnew[0:1, 0:HH2], in0=brow[0:1, 0:HH2], in1=cp0[:],
                            op=ALU.subtract,
                        )
                        nc.vector.tensor_tensor(
                            out=cnew[0:1, HH2:LR], in0=brow[0:1, HH2:LR], in1=cp1[:],
                            op=ALU.subtract,
                        )
                        nc.vector.tensor_scalar(
                            cnew[:], cnew[:], ivn[0:1, i : i + 1], None, ALU.mult
                        )
                    else:
                        nc.vector.tensor_scalar(
                            cnew[:], brow[:], ivn[0:1, i : i + 1], None, ALU.mult
                        )

                    # Kahan-compensated downdate: d -= c_new^2
                    # s = c2 + comp ; t = d - s ; r = d - t ; comp = s - r
                    nc.vector.tensor_tensor(out=c2[:], in0=cnew[:], in1=cnew[:], op=ALU.mult)
                    nc.vector.tensor_tensor(out=c2[:], in0=c2[:], in1=comp[:], op=ALU.add)
                    nc.vector.tensor_tensor(out=d_nxt[:], in0=d_cur[:], in1=c2[:], op=ALU.subtract)

                    # store column i of C (row i of C^T) before next gcol extract
                    nc.sync.dma_start(out=ct[i : i + 1, :], in_=cnew[:])

                    if i + 1 < K:
                        local_argmax(i + 1, d_nxt[:])

                    # Kahan scratch (only needed next iteration; hides under the AG)
                    nc.vector.tensor_tensor(out=brow[:], in0=d_cur[:], in1=d_nxt[:], op=ALU.subtract)
                    nc.vector.tensor_tensor(out=comp[:], in0=c2[:], in1=brow[:], op=ALU.subtract)

            # ---------------- Phase 3: forward substitution V = Lp^{-1} P ----------------
            # All row-i work happens at partition 0 (PE psum base must be 0/32/64);
            # finished rows are DMA'd to partition i of vsb for use as matmul rhs.
            with (
                tc.tile_pool(name="vps", bufs=2, space="PSUM") as vpool,
                tc.tile_pool(name="p3", bufs=4) as p3pool,
            ):
                for i in range(K):
                    prow = p3pool.tile([1, LC], F32, tag="prow")
                    nc.sync.dma_start(out=prow[:], in_=psb[i : i + 1, :])
                    vrow = p3pool.tile([1, LC], F32, tag="vrow")
                    if i > 0:
                        vps = vpool.tile([1, LC], F32, tag="vps")
                        nc.tensor.matmul(
                            vps[:], lhsT=lpt[:, i : i + 1], rhs=vsb[:],
                            start=True, stop=True,
                        )
                        nc.vector.tensor_tensor(
                            out=vrow[:], in0=prow[:], in1=vps[:], op=ALU.subtract
                        )
                        nc.vector.tensor_scalar(
                            vrow[:], vrow[:], ivn[0:1, i : i + 1], None, ALU.mult
                        )
                    else:
                        nc.vector.tensor_scalar(
                            vrow[:], prow[:], ivn[0:1, 0:1], None, ALU.mult
                        )
                    nc.sync.dma_start(out=vsb[i : i + 1, :], in_=vrow[:])

            if debug:
                nc.sync.dma_start(out=bdo[:], in_=bd[:, :])
                nc.sync.dma_start(out=cto[:], in_=ct[:])
            nc.sync.dma_start(out=vout[:], in_=vsb[:])
            nc.sync.dma_start(out=vnso[:], in_=vns[:])
            nc.sync.dma_start(out=pivo[:], in_=piv[:])

    nc.compile()
    return nc


_NC_CACHE = None


def _get_nc():
    global _NC_CACHE
    if _NC_CACHE is None:
        _NC_CACHE = _build()
    return _NC_CACHE


def run_device(x, nc=None, **kwargs):
    x = np.ascontiguousarray(x, dtype=np.float32)
    n, d = x.shape
    NCORE = 8
    LR = n // NCORE
    LC = d // NCORE
    xt = np.ascontiguousarray(x.T)
    KT, ML = d // 128, LR // 128
    d0 = np.einsum("ij,ij->i", x, x)
    in_maps = []
    for j in range(NCORE):
        blk = xt[:, j * LR : (j + 1) * LR]  # [d, LR]
        xttl = np.ascontiguousarray(
            blk.reshape(KT, 128, ML, 128).transpose(2, 0, 1, 3)
        ).reshape(ML * KT * 128, 128)
        in_maps.append(
            {
                "xttl": xttl,
                "xcols": np.ascontiguousarray(x[:, j * LC : (j + 1) * LC]),
                "d0": np.ascontiguousarray(d0[j * LR : (j + 1) * LR]).reshape(1, LR),
            }
        )
    if nc is None:
        nc = _get_nc()
    return run_bass_kernel_spmd(nc, in_maps, core_ids=list(range(NCORE)), **kwargs)


def kernel(x_diff):
    out = run_device(x_diff)
    res = out.results
    V = np.concatenate([res[j]["vout"] for j in range(8)], axis=1).astype(np.float32)
    vns = res[0]["vnso"].reshape(-1)
    broken = np.where(vns < 1e-6)[0]
    n_succ = int(broken[0]) if len(broken) else K
    if n_succ < K:
        V[n_succ:] = 0.0
    return V, np.int32(n_succ)
